# revision 9
# baseline (speedup 1.0000x reference)
"""Tensor-parallel GQA attention prefill (Llama-style) on one TRN2 chip.

Head-sharded across 8 NeuronCores: core c owns q-heads [4c, 4c+4) and
kv-head c.  x is replicated (pre-transposed on host), wq/wk/wv are
column-sharded, wo row-sharded; each core computes a partial output
[B*S, DIM] and the host sums the 8 partials.

Self-contained: shapes hardcoded for
  x[2,2048,4096] wq[4096,4096] wk/wv[1024,4096] wo[4096,4096]
  32 q heads / 8 kv heads / head_dim 128 / causal prefill (start_pos=0).
"""

import math

import numpy as np
import ml_dtypes

import concourse.bass as bass
import concourse.mybir as mybir
from concourse import bacc
from concourse.tile import TileContext
from concourse.bass_utils import run_bass_kernel_spmd
from concourse.masks import make_identity

BSZ, SEQ, DIM = 2, 2048, 4096
NH, NKV, HD = 32, 8, 128
NCORES = 8
HPC = NH // NCORES          # 4 q heads per core
BS = BSZ * SEQ              # 4096 flattened rows
NJ = BS // 512              # 8 s-chunks of 512
KT = DIM // 128             # 32 contraction tiles
SBLK = 4                    # 512-wide s-blocks per batch
BF16 = mybir.dt.bfloat16
FP8 = mybir.dt.float8e4
F32 = mybir.dt.float32
NPBF16 = ml_dtypes.bfloat16
NPF8 = ml_dtypes.float8_e4m3
ALU = mybir.AluOpType
AF = mybir.ActivationFunctionType
DR = mybir.MatmulPerfMode.DoubleRow

# power-of-2 pre-scales so fp8e4 operands sit mid-range (split residuals
# stay out of the subnormal zone); descaled on PSUM evacuation.
SX = 16.0                   # x (std 1)
SWQ = 8192.0                # wq/sqrt(HD) (std ~0.0018)
SWK = 1024.0                # wk (std 0.02)
SWV = 1024.0
DSQ = 1.0 / (SX * SWQ)
DSK = 1.0 / (SX * SWK)
DSV = 1.0 / (SX * SWV)


def build_graph():
    nc = bacc.Bacc("TRN2", target_bir_lowering=False)
    x1T = nc.declare_dram_parameter("x1T", [DIM, BS], FP8, isOutput=False)
    x2T = nc.declare_dram_parameter("x2T", [DIM, BS], FP8, isOutput=False)
    wq1T = nc.declare_dram_parameter("wq1T", [DIM, HPC * HD], FP8, isOutput=False)
    wq2T = nc.declare_dram_parameter("wq2T", [DIM, HPC * HD], FP8, isOutput=False)
    wk1T = nc.declare_dram_parameter("wk1T", [DIM, HD], FP8, isOutput=False)
    wk2T = nc.declare_dram_parameter("wk2T", [DIM, HD], FP8, isOutput=False)
    wv1T = nc.declare_dram_parameter("wv1T", [DIM, HD], FP8, isOutput=False)
    wv2T = nc.declare_dram_parameter("wv2T", [DIM, HD], FP8, isOutput=False)
    woT = nc.declare_dram_parameter("woT", [HPC * HD, DIM], BF16, isOutput=False)
    cosT = nc.declare_dram_parameter("cosT", [HD // 2, SEQ], BF16, isOutput=False)
    sinT = nc.declare_dram_parameter("sinT", [HD // 2, SEQ], BF16, isOutput=False)
    tri = nc.declare_dram_parameter("tri", [128, 128], BF16, isOutput=False)
    out = nc.declare_dram_parameter("out", [BS, DIM], BF16, isOutput=True)

    with TileContext(nc) as tc:
        with (
            tc.tile_pool(name="const", bufs=1) as const,
            tc.tile_pool(name="xtp", bufs=3) as xtp,
            tc.tile_pool(name="ropep", bufs=2) as ropep,
            tc.tile_pool(name="ptp", bufs=8) as ptp,
            tc.tile_pool(name="atp", bufs=3) as atp,
            tc.tile_pool(name="accp", bufs=2) as accp,
            tc.tile_pool(name="recp", bufs=2) as recp,
            tc.tile_pool(name="osb", bufs=3) as osb,
            tc.tile_pool(name="psA", bufs=4, space="PSUM") as psA,
            tc.tile_pool(name="psB", bufs=4, space="PSUM") as psB,
        ):
            # ---- resident constants / weights -------------------------------
            # wq/wk/wv are DMA'd per k-slice inside the j==0 loop so the
            # first matmuls start as soon as their slice lands.
            wq1_sb = const.tile([128, KT, HPC * HD], FP8, tag="wq1")
            wq2_sb = const.tile([128, KT, HPC * HD], FP8, tag="wq2")
            wk1_sb = const.tile([128, KT, HD], FP8, tag="wk1")
            wk2_sb = const.tile([128, KT, HD], FP8, tag="wk2")
            wv1_sb = const.tile([128, KT, HD], FP8, tag="wv1")
            wv2_sb = const.tile([128, KT, HD], FP8, tag="wv2")
            cos_sb = const.tile([64, SEQ], BF16, tag="cos")
            nc.sync.dma_start(cos_sb[:], cosT[:])
            sin_sb = const.tile([64, SEQ], BF16, tag="sin")
            nc.sync.dma_start(sin_sb[:], sinT[:])
            tri_sb = const.tile([128, 128], BF16, tag="tri")
            nc.sync.dma_start(tri_sb[:], tri[:])
            # wo is first needed in the attention phase; loaded there.
            wo_sb = const.tile([128, HPC, DIM], BF16, tag="wo")

            ones_sb = const.tile([128, 1], BF16, tag="ones")
            nc.gpsimd.memset(ones_sb[:], 1.0)
            ident = const.tile([128, 128], BF16, tag="ident")
            make_identity(nc, ident[:])

            # Preload the exp table so the first attention exp doesn't pay
            # the ACT_TABLE_LOAD, and run warm-up matmuls on ident during the
            # initial DMA window so HAM unthrottles before the real work.
            scr = const.tile([128, 1], BF16, tag="scr")
            nc.scalar.activation(scr[:], ones_sb[:], AF.Exp)
            warm = psA.tile([128, 512], F32, tag="psA", name="warm")
            for _ in range(130):
                nc.tensor.matmul(warm[:, 0:128], lhsT=ident[:], rhs=ident[:],
                                 start=True, stop=True)

            # ---- resident activations ---------------------------------------
            qT_sb = const.tile([128, HPC, BS], BF16, tag="qT")    # per-head Q^T
            kT_sb = const.tile([128, BS], BF16, tag="kT")         # K^T (d, t)
            v_sb = const.tile([128, BS // 128, HD], BF16, tag="v")  # V (t, d) tiles
            # attention output lives per-block in the atp pool (2 blocks live)

            def rope_pre(psum, ds):
                """Copy psum halves to SBUF (releases the PSUM slot),
                applying the fp8 descale."""
                te = ropep.tile([64, 512], BF16, tag="ropetmpe", bufs=5)
                to = ropep.tile([64, 512], BF16, tag="ropetmpo", bufs=5)
                nc.scalar.activation(te[:], psum[0:64], AF.Copy, scale=ds)
                nc.vector.tensor_scalar_mul(to[:], psum[64:128], ds)
                return te, to

            def rope_post(te, to, dst, soff):
                cs = cos_sb[:, soff:soff + 512]
                sn = sin_sb[:, soff:soff + 512]
                t1 = ropep.tile([64, 512], BF16, tag="t1")
                t2 = ropep.tile([64, 512], BF16, tag="t2")
                nc.vector.tensor_tensor(t1[:], te[:], cs, ALU.mult)
                nc.vector.tensor_tensor(t2[:], to[:], sn, ALU.mult)
                nc.vector.tensor_tensor(dst[0:64], t1[:], t2[:], ALU.subtract)
                t3 = ropep.tile([64, 512], BF16, tag="t1")
                t4 = ropep.tile([64, 512], BF16, tag="t2")
                nc.vector.tensor_tensor(t3[:], te[:], sn, ALU.mult)
                nc.vector.tensor_tensor(t4[:], to[:], cs, ALU.mult)
                nc.vector.tensor_tensor(dst[64:128], t3[:], t4[:], ALU.add)

            def rope_copy(psum, dst, soff, ds):
                """psum [128,512] (evens-first layout) -> rotated bf16 dst."""
                te, to = rope_pre(psum, ds)
                rope_post(te, to, dst, soff)

            deferred_rope = []

            # ================= Phase 1: QKV projection =======================
            # single pass over xT per s-chunk: 4 Q accumulators in psA,
            # K/V accumulators in psB.
            for j in range(NJ):
                soff = (j % SBLK) * 512      # within-batch s offset
                js = slice(j * 512, (j + 1) * 512)
                qps = [psA.tile([128, 512], F32, tag="psA", name=f"qps{j}_{c}") for c in range(HPC)]
                kp = psB.tile([128, 512], F32, tag="psB", name=f"kp{j}")
                vp = psB.tile([128, 512], F32, tag="psB", name=f"vp{j}")
                for kc in range(KT // 4):
                    x1t = xtp.tile([128, 4, 512], FP8, tag="x1t")
                    nc.sync.dma_start(
                        x1t[:],
                        x1T[kc * 512:(kc + 1) * 512, js].rearrange("(a p) m -> p a m", p=128))
                    x2t = xtp.tile([128, 4, 512], FP8, tag="x2t")
                    nc.sync.dma_start(
                        x2t[:],
                        x2T[kc * 512:(kc + 1) * 512, js].rearrange("(a p) m -> p a m", p=128))
                    if j == 0:
                        for k4 in range(4):
                            ks = slice((kc * 4 + k4) * 128, (kc * 4 + k4 + 1) * 128)
                            nc.sync.dma_start(wq1_sb[:, kc * 4 + k4, :], wq1T[ks, :])
                            nc.sync.dma_start(wq2_sb[:, kc * 4 + k4, :], wq2T[ks, :])
                            nc.sync.dma_start(wk1_sb[:, kc * 4 + k4, :], wk1T[ks, :])
                            nc.sync.dma_start(wk2_sb[:, kc * 4 + k4, :], wk2T[ks, :])
                            nc.sync.dma_start(wv1_sb[:, kc * 4 + k4, :], wv1T[ks, :])
                            nc.sync.dma_start(wv2_sb[:, kc * 4 + k4, :], wv2T[ks, :])
                    # fp8 DoubleRow: each instruction contracts a 256-row
                    # k-pair; 3 chains (w1x1 + w2x1 + w1x2) realize the
                    # error-compensated fp8 product.
                    for i2 in range(2):
                        k2 = kc * 2 + i2
                        first, last = k2 == 0, k2 == KT // 2 - 1
                        sl = slice(kc * 4 + 2 * i2, kc * 4 + 2 * i2 + 2)
                        x1s = x1t[:, 2 * i2:2 * i2 + 2, :]
                        x2s = x2t[:, 2 * i2:2 * i2 + 2, :]
                        for acc, w1_sb, w2_sb, cs in (
                            (kp, wk1_sb, wk2_sb, slice(0, HD)),
                            (vp, wv1_sb, wv2_sb, slice(0, HD)),
                        ):
                            nc.tensor.matmul(acc[:], lhsT=w1_sb[:, sl, cs], rhs=x1s,
                                             start=first, stop=False, perf_mode=DR)
                            nc.tensor.matmul(acc[:], lhsT=w2_sb[:, sl, cs], rhs=x1s,
                                             start=False, stop=False, perf_mode=DR)
                            nc.tensor.matmul(acc[:], lhsT=w1_sb[:, sl, cs], rhs=x2s,
                                             start=False, stop=last, perf_mode=DR)
                        for c in range(HPC):
                            cs = slice(c * 128, (c + 1) * 128)
                            nc.tensor.matmul(qps[c][:], lhsT=wq1_sb[:, sl, cs], rhs=x1s,
                                             start=first, stop=False, perf_mode=DR)
                            nc.tensor.matmul(qps[c][:], lhsT=wq2_sb[:, sl, cs], rhs=x1s,
                                             start=False, stop=False, perf_mode=DR)
                            nc.tensor.matmul(qps[c][:], lhsT=wq1_sb[:, sl, cs], rhs=x2s,
                                             start=False, stop=last, perf_mode=DR)
                # K/V first: attention needs them (and their PSUM slots) at the
                # phase boundary before any Q-rope results.
                rope_copy(kp, kT_sb[:, js], soff, DSK)
                # V^T chunk -> natural-layout V tiles via DMA XBAR transpose.
                # Last chunk's PSUM copy goes on DVE so the ScalarE queue is
                # clear for the first attention exp at the phase boundary.
                last = j == NJ - 1
                vtmp = ropep.tile([128, 512], BF16, tag="vtmp")
                if last:
                    nc.vector.tensor_scalar_mul(vtmp[:], vp[:], DSV)
                else:
                    nc.scalar.activation(vtmp[:], vp[:], AF.Copy, scale=DSV)
                nc.sync.dma_start_transpose(v_sb[:, j * 4:(j + 1) * 4, :], vtmp[:])
                for c in range(HPC):
                    if last:
                        # defer the DVE rope chains past the phase boundary;
                        # qT chunk 7 isn't read until the 7th attention block.
                        te, to = rope_pre(qps[c], DSQ)
                        deferred_rope.append((te, to, qT_sb[:, c, js], soff))
                    else:
                        rope_copy(qps[c], qT_sb[:, c, js], soff, DSQ)
                if j == 4:
                    # mid-phase: DMA bandwidth has headroom here and wo is
                    # needed right after the phase boundary.
                    nc.sync.dma_start(wo_sb[:], woT.rearrange("(a p) m -> p a m", p=128))

            # ================= Phase 2+3: attention + out-proj ===============
            # Out-proj of the previous block is interleaved at attention-tile
            # granularity so the PE never starves while ScalarE runs exp; the
            # den/recip chain of each head is deferred by one head so its
            # den-matmul never blocks the PE FIFO on the DVE accumulation.
            def outproj_group(b, s_lo, s_w, at_blk, g):
                st, n = g // 8, g % 8
                s0 = b * SEQ + s_lo + st * 128
                po = psA.tile([128, 512], F32, tag="psA", name=f"po{b}_{s_lo}_{g}")
                for dt in range(HPC):
                    nc.tensor.matmul(
                        po[:], lhsT=at_blk[:, dt, st * 128:(st + 1) * 128],
                        rhs=wo_sb[:, dt, n * 512:(n + 1) * 512],
                        start=(dt == 0), stop=(dt == HPC - 1))
                ob = osb.tile([128, 512], BF16, tag="ob")
                if n % 2 == 0:
                    nc.scalar.copy(ob[:], po[:])
                else:
                    nc.vector.tensor_copy(ob[:], po[:])
                nc.sync.dma_start(out[s0:s0 + 128, n * 512:(n + 1) * 512], ob[:])

            def den_chain(h, s_w, ppv, ptsum, at_blk, psc_last):
                # reuse the head's last (already-read) scores tile for the
                # denominator row instead of burning another PSUM slot
                pden = psc_last[0:1, 0:s_w]
                nc.tensor.matmul(pden, lhsT=ones_sb[:],
                                 rhs=ptsum[:, 0:s_w], start=True, stop=True)
                recf = recp.tile([1, 512], F32, tag="recf")
                nc.vector.reciprocal_approx_fast(recf[:, 0:s_w], pden)
                rec = recp.tile([1, 512], BF16, tag="rec")
                nc.vector.tensor_copy(rec[:, 0:s_w], recf[:, 0:s_w])
                rep = recp.tile([128, 512], BF16, tag="rep")
                nc.gpsimd.partition_broadcast(rep[:, 0:s_w], rec[:, 0:s_w])
                nc.vector.tensor_tensor(at_blk[:, h, 0:s_w],
                                        ppv[:, 0:s_w], rep[:, 0:s_w], ALU.mult)

            # (batch, s_lo, s_width); within a batch s ranges must ascend.
            # The final block is split in two halves so half the last
            # out-proj overlaps the second half's attention.
            blocks = [(0, 0, 512), (0, 512, 512), (0, 1024, 512), (0, 1536, 512),
                      (1, 512, 512), (1, 1024, 512), (1, 1536, 512),
                      (1, 0, 256), (1, 256, 256)]
            prev = None
            for bi, (b, s_lo, s_w) in enumerate(blocks):
                # NOTE: keep GpSimd to a single op type (partition_broadcast);
                # mixing in tensor_tensor forces ~5us microcode library swaps.
                acc_eng = nc.vector
                sg = b * SEQ + s_lo
                nt = (s_lo + s_w) // 128             # causal t-tiles
                ntile = HPC * nt
                ngrp_prev = (prev[2] // 128) * 8 if prev is not None else 0
                at_blk = atp.tile([128, HPC, 512], BF16, tag="attnT",
                                  name=f"attnT{sg}")
                emitted = 0
                tidx = 0
                pend = None
                for h in range(HPC):
                    ppv = psA.tile([128, 512], F32, tag="psA", name=f"ppv{sg}{h}")
                    ptsum = accp.tile([128, 512], BF16, tag="ptsum",
                                      name=f"ptsum{sg}{h}")
                    for ti in range(nt):
                        tg = b * SEQ + ti * 128
                        w0 = max(0, 128 * ti - s_lo)
                        diag = 128 * ti >= s_lo
                        psc = psB.tile([128, 512], F32, tag="psB",
                                       name=f"psc{sg}{h}{ti}")
                        nc.tensor.matmul(psc[:, w0:s_w],
                                         lhsT=kT_sb[:, tg:tg + 128],
                                         rhs=qT_sb[:, h, sg + w0:sg + s_w],
                                         start=True, stop=not diag)
                        if diag:                     # additive causal mask via PE
                            nc.tensor.matmul(psc[:, w0:w0 + 128], lhsT=ident[:],
                                             rhs=tri_sb[:],
                                             start=False, stop=True)
                        if ti == 0:
                            pt = ptsum               # exp seeds the running sum
                        else:
                            pt = ptp.tile([128, 512], BF16, tag="pt")
                        nc.scalar.activation(pt[:, w0:s_w], psc[:, w0:s_w], AF.Exp)
                        # out-proj of the previous block goes between scores
                        # and pv so the PE has work during the exp latency.
                        tidx += 1
                        if prev is not None:
                            want = tidx * ngrp_prev // ntile
                            while emitted < want:
                                outproj_group(*prev, emitted)
                                emitted += 1
                        if ti > 0:
                            acc_eng.tensor_tensor(ptsum[:, w0:s_w],
                                                  ptsum[:, w0:s_w],
                                                  pt[:, w0:s_w], ALU.add)
                        nc.tensor.matmul(ppv[:, w0:s_w],
                                         lhsT=v_sb[:, tg // 128, :],
                                         rhs=pt[:, w0:s_w], start=(ti == 0),
                                         stop=(ti == nt - 1))
                    if pend is not None:
                        den_chain(*pend)
                    pend = (h, s_w, ppv, ptsum, at_blk, psc)
                if prev is not None:
                    while emitted < ngrp_prev:
                        outproj_group(*prev, emitted)
                        emitted += 1
                den_chain(*pend)
                if deferred_rope:
                    rope_post(*deferred_rope.pop(0))
                prev = (b, s_lo, s_w, at_blk)
            for g in range((prev[2] // 128) * 8):
                outproj_group(*prev, g)
    nc.finalize()
    return nc


_GRAPH = None


def _get_graph():
    global _GRAPH
    if _GRAPH is None:
        _GRAPH = build_graph()
    return _GRAPH


def prepare_in_maps(x, wq, wk, wv, wo, freqs_cos, freqs_sin, mask, start_pos=0):
    x = np.asarray(x, np.float32)
    wq = np.asarray(wq, np.float32)
    wk = np.asarray(wk, np.float32)
    wv = np.asarray(wv, np.float32)
    wo = np.asarray(wo, np.float32)
    fc = np.asarray(freqs_cos, np.float32)
    fs = np.asarray(freqs_sin, np.float32)

    # evens-first pair permutation (interleaved rope -> rotate-half form)
    perm = np.concatenate([np.arange(0, HD, 2), np.arange(1, HD, 2)])

    def permute_heads(w):
        wr = w.reshape(-1, HD, DIM)[:, perm, :]
        return wr.reshape(-1, DIM)

    def split8(a, s):
        a1 = np.clip(a * s, -240, 240).astype(NPF8)
        a2 = np.clip(a * s - a1.astype(np.float32), -240, 240).astype(NPF8)
        return a1, a2

    wq_p = permute_heads(wq) * (1.0 / math.sqrt(HD))
    wk_p = permute_heads(wk)

    xS = np.ascontiguousarray(x.reshape(BS, DIM).T)
    x1T, x2T = split8(xS, SX)
    cosT = np.ascontiguousarray(fc.T).astype(NPBF16)
    sinT = np.ascontiguousarray(fs.T).astype(NPBF16)
    # additive causal triangle for the in-tile diagonal: tri[t, c] = 0 if
    # c >= t else -1e9 (c = column within the 128-wide diagonal strip)
    tt, cc = np.meshgrid(np.arange(128), np.arange(128), indexing="ij")
    tri = np.where(cc >= tt, 0.0, -1e9).astype(NPBF16)

    in_maps = []
    for c in range(NCORES):
        qs = slice(c * HPC * HD, (c + 1) * HPC * HD)
        ks = slice(c * HD, (c + 1) * HD)
        wq1, wq2 = split8(np.ascontiguousarray(wq_p[qs, :].T), SWQ)
        wk1, wk2 = split8(np.ascontiguousarray(wk_p[ks, :].T), SWK)
        wv1, wv2 = split8(np.ascontiguousarray(wv[ks, :].T), SWV)
        in_maps.append({
            "x1T": x1T,
            "x2T": x2T,
            "wq1T": wq1, "wq2T": wq2,
            "wk1T": wk1, "wk2T": wk2,
            "wv1T": wv1, "wv2T": wv2,
            "woT": np.ascontiguousarray(wo[:, qs].T).astype(NPBF16),
            "cosT": cosT,
            "sinT": sinT,
            "tri": tri,
        })
    return in_maps


def combine_results(results):
    acc = results[0]["out"].astype(np.float64)
    for c in range(1, NCORES):
        acc = acc + results[c]["out"]
    return acc.astype(np.float32).reshape(BSZ, SEQ, DIM)


def run_spmd(in_maps, **kw):
    nc = _get_graph()
    return run_bass_kernel_spmd(nc, in_maps, list(range(NCORES)), **kw)


def kernel(x, wq, wk, wv, wo, freqs_cos, freqs_sin, mask, start_pos=0, **_):
    in_maps = prepare_in_maps(x, wq, wk, wv, wo, freqs_cos, freqs_sin, mask)
    res = run_spmd(in_maps)
    return combine_results(res.results)



# revision 10
# speedup vs baseline: 1.2359x; 1.2359x over previous
"""Tensor-parallel GQA attention prefill (Llama-style) on one TRN2 chip.

Head-sharded across 8 NeuronCores: core c owns q-heads [4c, 4c+4) and
kv-head c.  x is replicated (pre-transposed on host), wq/wk/wv are
column-sharded, wo row-sharded; each core computes a partial output
[B*S, DIM] and the host sums the 8 partials.

Self-contained: shapes hardcoded for
  x[2,2048,4096] wq[4096,4096] wk/wv[1024,4096] wo[4096,4096]
  32 q heads / 8 kv heads / head_dim 128 / causal prefill (start_pos=0).
"""

import math

import numpy as np
import ml_dtypes

import concourse.bass as bass
import concourse.mybir as mybir
from concourse import bacc
from concourse.tile import TileContext
from concourse.bass_utils import run_bass_kernel_spmd
from concourse.masks import make_identity

BSZ, SEQ, DIM = 2, 2048, 4096
NH, NKV, HD = 32, 8, 128
NCORES = 8
HPC = NH // NCORES          # 4 q heads per core
BS = BSZ * SEQ              # 4096 flattened rows
NJ = BS // 512              # 8 s-chunks of 512
KT = DIM // 128             # 32 contraction tiles
SBLK = 4                    # 512-wide s-blocks per batch
QLAG = 4                    # Q chains trail K/V by this many k-steps
BF16 = mybir.dt.bfloat16
F32 = mybir.dt.float32
NPBF16 = ml_dtypes.bfloat16
ALU = mybir.AluOpType
AF = mybir.ActivationFunctionType


def build_graph():
    nc = bacc.Bacc("TRN2", target_bir_lowering=False)
    xT = nc.declare_dram_parameter("xT", [DIM, BS], BF16, isOutput=False)
    wqT = nc.declare_dram_parameter("wqT", [DIM, HPC * HD], BF16, isOutput=False)
    wkT = nc.declare_dram_parameter("wkT", [DIM, HD], BF16, isOutput=False)
    wvT = nc.declare_dram_parameter("wvT", [DIM, HD], BF16, isOutput=False)
    woT = nc.declare_dram_parameter("woT", [HPC * HD, DIM], BF16, isOutput=False)
    cosT = nc.declare_dram_parameter("cosT", [HD // 2, SEQ], BF16, isOutput=False)
    sinT = nc.declare_dram_parameter("sinT", [HD // 2, SEQ], BF16, isOutput=False)
    tri = nc.declare_dram_parameter("tri", [128, 128], F32, isOutput=False)
    out = nc.declare_dram_parameter("out", [BS, DIM], BF16, isOutput=True)

    with TileContext(nc) as tc:
        with (
            tc.tile_pool(name="const", bufs=1) as const,
            tc.tile_pool(name="xtp", bufs=3) as xtp,
            tc.tile_pool(name="ropep", bufs=2) as ropep,
            tc.tile_pool(name="ptp", bufs=8) as ptp,
            tc.tile_pool(name="atp", bufs=3) as atp,
            tc.tile_pool(name="accp", bufs=2) as accp,
            tc.tile_pool(name="recp", bufs=2) as recp,
            tc.tile_pool(name="osb", bufs=3) as osb,
            tc.tile_pool(name="psA", bufs=4, space="PSUM") as psA,
            tc.tile_pool(name="psB", bufs=4, space="PSUM") as psB,
        ):
            # ---- resident constants / weights -------------------------------
            # wq/wk/wv are DMA'd per k-slice inside the j==0 loop so the
            # first matmuls start as soon as their slice lands.
            wq_sb = const.tile([128, KT, HPC * HD], BF16, tag="wq")
            wk_sb = const.tile([128, KT, HD], BF16, tag="wk")
            wv_sb = const.tile([128, KT, HD], BF16, tag="wv")
            cos_sb = const.tile([64, SEQ], BF16, tag="cos")
            nc.sync.dma_start(cos_sb[:], cosT[:])
            sin_sb = const.tile([64, SEQ], BF16, tag="sin")
            nc.sync.dma_start(sin_sb[:], sinT[:])
            tri_sb = const.tile([128, 128], F32, tag="tri")
            nc.sync.dma_start(tri_sb[:], tri[:])
            # wo is first needed in the attention phase; loaded there.
            wo_sb = const.tile([128, HPC, DIM], BF16, tag="wo")

            ones_sb = const.tile([128, 1], BF16, tag="ones")
            nc.gpsimd.memset(ones_sb[:], 1.0)
            ident = const.tile([128, 128], BF16, tag="ident")
            make_identity(nc, ident[:])

            # Preload the exp table so the first attention exp doesn't pay
            # the ACT_TABLE_LOAD, and run warm-up matmuls on ident during the
            # initial DMA window so HAM unthrottles before the real work.
            scr = const.tile([128, 1], BF16, tag="scr")
            nc.scalar.activation(scr[:], ones_sb[:], AF.Exp)
            warm = psA.tile([128, 512], F32, tag="psA", name="warm")
            for _ in range(130):
                nc.tensor.matmul(warm[:, 0:128], lhsT=ident[:], rhs=ident[:],
                                 start=True, stop=True)

            # ---- resident activations ---------------------------------------
            qT_sb = const.tile([128, HPC, BS], BF16, tag="qT")    # per-head Q^T
            kT_sb = const.tile([128, BS], BF16, tag="kT")         # K^T (d, t)
            v_sb = const.tile([128, BS // 128, HD], BF16, tag="v")  # V (t, d) tiles
            # attention output lives per-block in the atp pool (2 blocks live)

            def rope_pre(psum):
                """Copy psum halves to SBUF (releases the PSUM slot)."""
                te = ropep.tile([64, 512], BF16, tag="ropetmpe", bufs=5)
                to = ropep.tile([64, 512], BF16, tag="ropetmpo", bufs=5)
                nc.scalar.copy(te[:], psum[0:64])
                nc.vector.tensor_copy(to[:], psum[64:128])
                return te, to

            def rope_post(te, to, dst, soff):
                cs = cos_sb[:, soff:soff + 512]
                sn = sin_sb[:, soff:soff + 512]
                t1 = ropep.tile([64, 512], BF16, tag="t1")
                t2 = ropep.tile([64, 512], BF16, tag="t2")
                nc.vector.tensor_tensor(t1[:], te[:], cs, ALU.mult)
                nc.vector.tensor_tensor(t2[:], to[:], sn, ALU.mult)
                nc.vector.tensor_tensor(dst[0:64], t1[:], t2[:], ALU.subtract)
                t3 = ropep.tile([64, 512], BF16, tag="t1")
                t4 = ropep.tile([64, 512], BF16, tag="t2")
                nc.vector.tensor_tensor(t3[:], te[:], sn, ALU.mult)
                nc.vector.tensor_tensor(t4[:], to[:], cs, ALU.mult)
                nc.vector.tensor_tensor(dst[64:128], t3[:], t4[:], ALU.add)

            def rope_copy(psum, dst, soff):
                """psum [128,512] (evens-first layout) -> rotated bf16 dst."""
                te, to = rope_pre(psum)
                rope_post(te, to, dst, soff)

            deferred_rope = []

            # ================= Phase 1: QKV projection =======================
            # single pass over xT per s-chunk: 4 Q accumulators in psA,
            # K/V accumulators in psB.  The Q chains trail K/V by QLAG
            # k-steps so at chunk boundaries the K/V chains of chunk j+1
            # start while chunk j's Q rope copies drain the psA banks.
            for j in range(NJ):
                soff = (j % SBLK) * 512      # within-batch s offset
                js = slice(j * 512, (j + 1) * 512)
                qps = [psA.tile([128, 512], F32, tag="psA", name=f"qps{j}_{c}") for c in range(HPC)]
                kp = psB.tile([128, 512], F32, tag="psB", name=f"kp{j}")
                vp = psB.tile([128, 512], F32, tag="psB", name=f"vp{j}")
                xts = {}

                def q_step(k):
                    kc4, k4 = divmod(k, 4)
                    for c in range(HPC):
                        nc.tensor.matmul(
                            qps[c][:], lhsT=wq_sb[:, k, c * 128:(c + 1) * 128],
                            rhs=xts[kc4][:, k4, :], start=(k == 0),
                            stop=(k == KT - 1))

                for kc in range(KT // 4):
                    xt = xtp.tile([128, 4, 512], BF16, tag="xt")
                    nc.sync.dma_start(
                        xt[:],
                        xT[kc * 512:(kc + 1) * 512, js].rearrange("(a p) m -> p a m", p=128))
                    xts[kc] = xt
                    if j == 0:
                        for k4 in range(4):
                            ks = slice((kc * 4 + k4) * 128, (kc * 4 + k4 + 1) * 128)
                            nc.sync.dma_start(wq_sb[:, kc * 4 + k4, :], wqT[ks, :])
                            nc.sync.dma_start(wk_sb[:, kc * 4 + k4, :], wkT[ks, :])
                            nc.sync.dma_start(wv_sb[:, kc * 4 + k4, :], wvT[ks, :])
                    for k4 in range(4):
                        k = kc * 4 + k4
                        nc.tensor.matmul(kp[:], lhsT=wk_sb[:, k, :], rhs=xt[:, k4, :],
                                         start=(k == 0), stop=(k == KT - 1))
                        nc.tensor.matmul(vp[:], lhsT=wv_sb[:, k, :], rhs=xt[:, k4, :],
                                         start=(k == 0), stop=(k == KT - 1))
                        if k >= QLAG:
                            q_step(k - QLAG)
                for k in range(KT - QLAG, KT):
                    q_step(k)
                # K/V first: attention needs them (and their PSUM slots) at the
                # phase boundary before any Q-rope results.
                rope_copy(kp, kT_sb[:, js], soff)
                # V^T chunk -> natural-layout V tiles via DMA XBAR transpose.
                # Last chunk's PSUM copy goes on DVE so the ScalarE queue is
                # clear for the first attention exp at the phase boundary.
                last = j == NJ - 1
                vtmp = ropep.tile([128, 512], BF16, tag="vtmp")
                if last:
                    nc.vector.tensor_copy(vtmp[:], vp[:])
                else:
                    nc.scalar.copy(vtmp[:], vp[:])
                nc.sync.dma_start_transpose(v_sb[:, j * 4:(j + 1) * 4, :], vtmp[:])
                for c in range(HPC):
                    if last:
                        # defer the DVE rope chains past the phase boundary;
                        # qT chunk 7 isn't read until the 7th attention block.
                        te, to = rope_pre(qps[c])
                        deferred_rope.append((te, to, qT_sb[:, c, js], soff))
                    else:
                        rope_copy(qps[c], qT_sb[:, c, js], soff)
                if j == 4:
                    # mid-phase: DMA bandwidth has headroom here and wo is
                    # needed right after the phase boundary.
                    nc.sync.dma_start(wo_sb[:], woT.rearrange("(a p) m -> p a m", p=128))

            # ================= Phase 2+3: attention + out-proj ===============
            # Out-proj of the previous block is interleaved at attention-tile
            # granularity so the PE never starves while ScalarE runs exp; the
            # den/recip chain of each head is deferred by one head so its
            # den-matmul never blocks the PE FIFO on the DVE accumulation.
            def outproj_group(b, s_lo, s_w, at_blk, g):
                st, n = g // 8, g % 8
                s0 = b * SEQ + s_lo + st * 128
                po = psA.tile([128, 512], F32, tag="psA", name=f"po{b}_{s_lo}_{g}")
                for dt in range(HPC):
                    nc.tensor.matmul(
                        po[:], lhsT=at_blk[:, dt, st * 128:(st + 1) * 128],
                        rhs=wo_sb[:, dt, n * 512:(n + 1) * 512],
                        start=(dt == 0), stop=(dt == HPC - 1))
                ob = osb.tile([128, 512], BF16, tag="ob")
                if n % 2 == 0:
                    nc.scalar.copy(ob[:], po[:])
                else:
                    nc.vector.tensor_copy(ob[:], po[:])
                nc.sync.dma_start(out[s0:s0 + 128, n * 512:(n + 1) * 512], ob[:])

            def den_chain(h, s_w, ppv, ptsum, at_blk, psc_last):
                # reuse the head's last (already-read) scores tile for the
                # denominator row instead of burning another PSUM slot
                pden = psc_last[0:1, 0:s_w]
                nc.tensor.matmul(pden, lhsT=ones_sb[:],
                                 rhs=ptsum[:, 0:s_w], start=True, stop=True)
                recf = recp.tile([1, 512], F32, tag="recf")
                nc.vector.reciprocal_approx_fast(recf[:, 0:s_w], pden)
                rec = recp.tile([1, 512], BF16, tag="rec")
                nc.vector.tensor_copy(rec[:, 0:s_w], recf[:, 0:s_w])
                rep = recp.tile([128, 512], BF16, tag="rep")
                nc.gpsimd.partition_broadcast(rep[:, 0:s_w], rec[:, 0:s_w])
                nc.vector.tensor_tensor(at_blk[:, h, 0:s_w],
                                        ppv[:, 0:s_w], rep[:, 0:s_w], ALU.mult)

            # (batch, s_lo, s_width); within a batch s ranges must ascend.
            # The final block is split in two halves so half the last
            # out-proj overlaps the second half's attention.
            blocks = [(0, 0, 512), (0, 512, 512), (0, 1024, 512), (0, 1536, 512),
                      (1, 512, 512), (1, 1024, 512), (1, 1536, 512),
                      (1, 0, 256), (1, 256, 256)]
            prev = None
            for bi, (b, s_lo, s_w) in enumerate(blocks):
                # NOTE: keep GpSimd to a single op type (partition_broadcast);
                # mixing in tensor_tensor forces ~5us microcode library swaps.
                acc_eng = nc.vector
                sg = b * SEQ + s_lo
                nt = (s_lo + s_w) // 128             # causal t-tiles
                ntile = HPC * nt
                ngrp_prev = (prev[2] // 128) * 8 if prev is not None else 0
                at_blk = atp.tile([128, HPC, 512], BF16, tag="attnT",
                                  name=f"attnT{sg}")
                emitted = 0
                tidx = 0
                pend = None
                for h in range(HPC):
                    ppv = psA.tile([128, 512], F32, tag="psA", name=f"ppv{sg}{h}")
                    ptsum = accp.tile([128, 512], BF16, tag="ptsum",
                                      name=f"ptsum{sg}{h}")
                    for ti in range(nt):
                        tg = b * SEQ + ti * 128
                        w0 = max(0, 128 * ti - s_lo)
                        diag = 128 * ti >= s_lo
                        psc = psB.tile([128, 512], F32, tag="psB",
                                       name=f"psc{sg}{h}{ti}")
                        nc.tensor.matmul(psc[:, w0:s_w],
                                         lhsT=kT_sb[:, tg:tg + 128],
                                         rhs=qT_sb[:, h, sg + w0:sg + s_w],
                                         start=True, stop=True)
                        if diag:                     # additive causal mask on DVE
                            nc.vector.tensor_tensor(psc[:, w0:w0 + 128],
                                                    psc[:, w0:w0 + 128],
                                                    tri_sb[:], ALU.add)
                        if ti == 0:
                            pt = ptsum               # exp seeds the running sum
                        else:
                            pt = ptp.tile([128, 512], BF16, tag="pt")
                        nc.scalar.activation(pt[:, w0:s_w], psc[:, w0:s_w], AF.Exp)
                        # out-proj of the previous block goes between scores
                        # and pv so the PE has work during the exp latency.
                        tidx += 1
                        if prev is not None:
                            want = tidx * ngrp_prev // ntile
                            while emitted < want:
                                outproj_group(*prev, emitted)
                                emitted += 1
                        if ti > 0:
                            acc_eng.tensor_tensor(ptsum[:, w0:s_w],
                                                  ptsum[:, w0:s_w],
                                                  pt[:, w0:s_w], ALU.add)
                        nc.tensor.matmul(ppv[:, w0:s_w],
                                         lhsT=v_sb[:, tg // 128, :],
                                         rhs=pt[:, w0:s_w], start=(ti == 0),
                                         stop=(ti == nt - 1))
                    if pend is not None:
                        den_chain(*pend)
                    pend = (h, s_w, ppv, ptsum, at_blk, psc)
                if prev is not None:
                    while emitted < ngrp_prev:
                        outproj_group(*prev, emitted)
                        emitted += 1
                den_chain(*pend)
                if deferred_rope:
                    rope_post(*deferred_rope.pop(0))
                prev = (b, s_lo, s_w, at_blk)
            for g in range((prev[2] // 128) * 8):
                outproj_group(*prev, g)
    nc.finalize()
    return nc


_GRAPH = None


def _get_graph():
    global _GRAPH
    if _GRAPH is None:
        _GRAPH = build_graph()
    return _GRAPH


def prepare_in_maps(x, wq, wk, wv, wo, freqs_cos, freqs_sin, mask, start_pos=0):
    x = np.asarray(x, np.float32)
    wq = np.asarray(wq, np.float32)
    wk = np.asarray(wk, np.float32)
    wv = np.asarray(wv, np.float32)
    wo = np.asarray(wo, np.float32)
    fc = np.asarray(freqs_cos, np.float32)
    fs = np.asarray(freqs_sin, np.float32)

    # evens-first pair permutation (interleaved rope -> rotate-half form)
    perm = np.concatenate([np.arange(0, HD, 2), np.arange(1, HD, 2)])

    def permute_heads(w):
        wr = w.reshape(-1, HD, DIM)[:, perm, :]
        return wr.reshape(-1, DIM)

    wq_p = permute_heads(wq) * (1.0 / math.sqrt(HD))
    wk_p = permute_heads(wk)

    xT = np.ascontiguousarray(x.reshape(BS, DIM).T).astype(NPBF16)
    cosT = np.ascontiguousarray(fc.T).astype(NPBF16)
    sinT = np.ascontiguousarray(fs.T).astype(NPBF16)
    # additive causal triangle for the in-tile diagonal: tri[t, c] = 0 if
    # c >= t else -1e9 (c = column within the 128-wide diagonal strip)
    tt, cc = np.meshgrid(np.arange(128), np.arange(128), indexing="ij")
    tri = np.where(cc >= tt, 0.0, -1e9).astype(np.float32)

    in_maps = []
    for c in range(NCORES):
        qs = slice(c * HPC * HD, (c + 1) * HPC * HD)
        ks = slice(c * HD, (c + 1) * HD)
        in_maps.append({
            "xT": xT,
            "wqT": np.ascontiguousarray(wq_p[qs, :].T).astype(NPBF16),
            "wkT": np.ascontiguousarray(wk_p[ks, :].T).astype(NPBF16),
            "wvT": np.ascontiguousarray(wv[ks, :].T).astype(NPBF16),
            "woT": np.ascontiguousarray(wo[:, qs].T).astype(NPBF16),
            "cosT": cosT,
            "sinT": sinT,
            "tri": tri,
        })
    return in_maps


def combine_results(results):
    acc = results[0]["out"].astype(np.float64)
    for c in range(1, NCORES):
        acc = acc + results[c]["out"]
    return acc.astype(np.float32).reshape(BSZ, SEQ, DIM)


def run_spmd(in_maps, **kw):
    nc = _get_graph()
    return run_bass_kernel_spmd(nc, in_maps, list(range(NCORES)), **kw)


def kernel(x, wq, wk, wv, wo, freqs_cos, freqs_sin, mask, start_pos=0, **_):
    in_maps = prepare_in_maps(x, wq, wk, wv, wo, freqs_cos, freqs_sin, mask)
    res = run_spmd(in_maps)
    return combine_results(res.results)


# revision 13
# speedup vs baseline: 1.2409x; 1.0040x over previous
"""Tensor-parallel GQA attention prefill (Llama-style) on one TRN2 chip.

Head-sharded across 8 NeuronCores: core c owns q-heads [4c, 4c+4) and
kv-head c.  x is replicated (pre-transposed on host), wq/wk/wv are
column-sharded, wo row-sharded; each core computes a partial output
[B*S, DIM] and the host sums the 8 partials.

Self-contained: shapes hardcoded for
  x[2,2048,4096] wq[4096,4096] wk/wv[1024,4096] wo[4096,4096]
  32 q heads / 8 kv heads / head_dim 128 / causal prefill (start_pos=0).
"""

import math

import numpy as np
import ml_dtypes

import concourse.bass as bass
import concourse.mybir as mybir
from concourse import bacc
from concourse.tile import TileContext
from concourse.bass_utils import run_bass_kernel_spmd
from concourse.masks import make_identity

BSZ, SEQ, DIM = 2, 2048, 4096
NH, NKV, HD = 32, 8, 128
NCORES = 8
HPC = NH // NCORES          # 4 q heads per core
BS = BSZ * SEQ              # 4096 flattened rows
NJ = BS // 512              # 8 s-chunks of 512
KT = DIM // 128             # 32 contraction tiles
SBLK = 4                    # 512-wide s-blocks per batch
BF16 = mybir.dt.bfloat16
F32 = mybir.dt.float32
NPBF16 = ml_dtypes.bfloat16
ALU = mybir.AluOpType
AF = mybir.ActivationFunctionType


def build_graph():
    nc = bacc.Bacc("TRN2", target_bir_lowering=False)
    xT = nc.declare_dram_parameter("xT", [DIM, BS], BF16, isOutput=False)
    wqT = nc.declare_dram_parameter("wqT", [DIM, HPC * HD], BF16, isOutput=False)
    wkT = nc.declare_dram_parameter("wkT", [DIM, HD], BF16, isOutput=False)
    wvT = nc.declare_dram_parameter("wvT", [DIM, HD], BF16, isOutput=False)
    woT = nc.declare_dram_parameter("woT", [HPC * HD, DIM], BF16, isOutput=False)
    cosT = nc.declare_dram_parameter("cosT", [HD // 2, SEQ], BF16, isOutput=False)
    sinT = nc.declare_dram_parameter("sinT", [HD // 2, SEQ], BF16, isOutput=False)
    tri = nc.declare_dram_parameter("tri", [128, 128], BF16, isOutput=False)
    out = nc.declare_dram_parameter("out", [BS, DIM], BF16, isOutput=True)

    with TileContext(nc) as tc:
        with (
            tc.tile_pool(name="const", bufs=1) as const,
            tc.tile_pool(name="xtp", bufs=3) as xtp,
            tc.tile_pool(name="ropep", bufs=2) as ropep,
            tc.tile_pool(name="ptp", bufs=8) as ptp,
            tc.tile_pool(name="atp", bufs=3) as atp,
            tc.tile_pool(name="accp", bufs=2) as accp,
            tc.tile_pool(name="recp", bufs=2) as recp,
            tc.tile_pool(name="osb", bufs=3) as osb,
            tc.tile_pool(name="psA", bufs=4, space="PSUM") as psA,
            tc.tile_pool(name="psB", bufs=4, space="PSUM") as psB,
        ):
            # ---- resident constants / weights -------------------------------
            # wq/wk/wv are DMA'd per k-slice inside the j==0 loop so the
            # first matmuls start as soon as their slice lands.
            wq_sb = const.tile([128, KT, HPC * HD], BF16, tag="wq")
            wk_sb = const.tile([128, KT, HD], BF16, tag="wk")
            wv_sb = const.tile([128, KT, HD], BF16, tag="wv")
            cos_sb = const.tile([64, SEQ], BF16, tag="cos")
            nc.sync.dma_start(cos_sb[:], cosT[:])
            sin_sb = const.tile([64, SEQ], BF16, tag="sin")
            nc.sync.dma_start(sin_sb[:], sinT[:])
            tri_sb = const.tile([128, 128], BF16, tag="tri")
            nc.sync.dma_start(tri_sb[:], tri[:])
            # wo is first needed in the attention phase; loaded there.
            wo_sb = const.tile([128, HPC, DIM], BF16, tag="wo")

            ones_sb = const.tile([128, 1], BF16, tag="ones")
            nc.gpsimd.memset(ones_sb[:], 1.0)
            ident = const.tile([128, 128], BF16, tag="ident")
            make_identity(nc, ident[:])

            # Preload the exp table so the first attention exp doesn't pay
            # the ACT_TABLE_LOAD, and run warm-up matmuls on ident during the
            # initial DMA window so HAM unthrottles before the real work.
            scr = const.tile([128, 1], BF16, tag="scr")
            nc.scalar.activation(scr[:], ones_sb[:], AF.Exp)
            warm = psA.tile([128, 512], F32, tag="psA", name="warm")
            for _ in range(130):
                nc.tensor.matmul(warm[:, 0:128], lhsT=ident[:], rhs=ident[:],
                                 start=True, stop=True)

            # ---- resident activations ---------------------------------------
            qT_sb = const.tile([128, HPC, BS], BF16, tag="qT")    # per-head Q^T
            kT_sb = const.tile([128, BS], BF16, tag="kT")         # K^T (d, t)
            v_sb = const.tile([128, BS // 128, HD], BF16, tag="v")  # V (t, d) tiles
            # attention output lives per-block in the atp pool (2 blocks live)

            def rope_pre(psum):
                """Copy psum halves to SBUF (releases the PSUM slot)."""
                te = ropep.tile([64, 512], BF16, tag="ropetmpe", bufs=5)
                to = ropep.tile([64, 512], BF16, tag="ropetmpo", bufs=5)
                nc.scalar.copy(te[:], psum[0:64])
                nc.vector.tensor_copy(to[:], psum[64:128])
                return te, to

            def rope_post(te, to, dst, soff):
                cs = cos_sb[:, soff:soff + 512]
                sn = sin_sb[:, soff:soff + 512]
                t1 = ropep.tile([64, 512], BF16, tag="t1")
                t2 = ropep.tile([64, 512], BF16, tag="t2")
                nc.vector.tensor_tensor(t1[:], te[:], cs, ALU.mult)
                nc.vector.tensor_tensor(t2[:], to[:], sn, ALU.mult)
                nc.vector.tensor_tensor(dst[0:64], t1[:], t2[:], ALU.subtract)
                t3 = ropep.tile([64, 512], BF16, tag="t1")
                t4 = ropep.tile([64, 512], BF16, tag="t2")
                nc.vector.tensor_tensor(t3[:], te[:], sn, ALU.mult)
                nc.vector.tensor_tensor(t4[:], to[:], cs, ALU.mult)
                nc.vector.tensor_tensor(dst[64:128], t3[:], t4[:], ALU.add)

            def rope_copy(psum, dst, soff):
                """psum [128,512] (evens-first layout) -> rotated bf16 dst."""
                te, to = rope_pre(psum)
                rope_post(te, to, dst, soff)

            deferred_rope = []

            # ================= Phase 1: QKV projection =======================
            # single pass over xT per s-chunk: 4 Q accumulators in psA,
            # K/V accumulators in psB.  The Q chains trail K/V by QLAG
            # k-steps so at chunk boundaries the K/V chains of chunk j+1
            # start while chunk j's Q rope copies drain the psA banks.
            for j in range(NJ):
                soff = (j % SBLK) * 512      # within-batch s offset
                js = slice(j * 512, (j + 1) * 512)
                qps = [psA.tile([128, 512], F32, tag="psA", name=f"qps{j}_{c}") for c in range(HPC)]
                kp = psB.tile([128, 512], F32, tag="psB", name=f"kp{j}")
                vp = psB.tile([128, 512], F32, tag="psB", name=f"vp{j}")
                xts = {}

                for kc in range(KT // 4):
                    xt = xtp.tile([128, 4, 512], BF16, tag="xt")
                    nc.sync.dma_start(
                        xt[:],
                        xT[kc * 512:(kc + 1) * 512, js].rearrange("(a p) m -> p a m", p=128))
                    if j == 0:
                        for k4 in range(4):
                            ks = slice((kc * 4 + k4) * 128, (kc * 4 + k4 + 1) * 128)
                            nc.sync.dma_start(wq_sb[:, kc * 4 + k4, :], wqT[ks, :])
                            nc.sync.dma_start(wk_sb[:, kc * 4 + k4, :], wkT[ks, :])
                            nc.sync.dma_start(wv_sb[:, kc * 4 + k4, :], wvT[ks, :])
                    for k4 in range(4):
                        k = kc * 4 + k4
                        nc.tensor.matmul(kp[:], lhsT=wk_sb[:, k, :], rhs=xt[:, k4, :],
                                         start=(k == 0), stop=(k == KT - 1))
                        nc.tensor.matmul(vp[:], lhsT=wv_sb[:, k, :], rhs=xt[:, k4, :],
                                         start=(k == 0), stop=(k == KT - 1))
                        for c in range(HPC):
                            nc.tensor.matmul(
                                qps[c][:], lhsT=wq_sb[:, k, c * 128:(c + 1) * 128],
                                rhs=xt[:, k4, :], start=(k == 0), stop=(k == KT - 1))
                # K/V first: attention needs them (and their PSUM slots) at the
                # phase boundary before any Q-rope results.
                rope_copy(kp, kT_sb[:, js], soff)
                # V^T chunk -> natural-layout V tiles via DMA XBAR transpose.
                # Last chunk's PSUM copy goes on DVE so the ScalarE queue is
                # clear for the first attention exp at the phase boundary.
                last = j == NJ - 1
                vtmp = ropep.tile([128, 512], BF16, tag="vtmp")
                if last:
                    nc.vector.tensor_copy(vtmp[:], vp[:])
                else:
                    nc.scalar.copy(vtmp[:], vp[:])
                nc.sync.dma_start_transpose(v_sb[:, j * 4:(j + 1) * 4, :], vtmp[:])
                for c in range(HPC):
                    if last:
                        # defer the DVE rope chains past the phase boundary;
                        # qT chunk 7 isn't read until the 7th attention block.
                        te, to = rope_pre(qps[c])
                        deferred_rope.append((te, to, qT_sb[:, c, js], soff))
                    else:
                        rope_copy(qps[c], qT_sb[:, c, js], soff)
                if j == 4:
                    # mid-phase: DMA bandwidth has headroom here and wo is
                    # needed right after the phase boundary.
                    nc.sync.dma_start(wo_sb[:], woT.rearrange("(a p) m -> p a m", p=128))

            # ================= Phase 2+3: attention + out-proj ===============
            # Out-proj of the previous block is interleaved at attention-tile
            # granularity so the PE never starves while ScalarE runs exp; the
            # den/recip chain of each head is deferred by one head so its
            # den-matmul never blocks the PE FIFO on the DVE accumulation.
            def outproj_group(b, s_lo, s_w, at_blk, g):
                st, n = g // 8, g % 8
                s0 = b * SEQ + s_lo + st * 128
                po = psA.tile([128, 512], F32, tag="psA", name=f"po{b}_{s_lo}_{g}")
                for dt in range(HPC):
                    nc.tensor.matmul(
                        po[:], lhsT=at_blk[:, dt, st * 128:(st + 1) * 128],
                        rhs=wo_sb[:, dt, n * 512:(n + 1) * 512],
                        start=(dt == 0), stop=(dt == HPC - 1))
                ob = osb.tile([128, 512], BF16, tag="ob")
                if n % 2 == 0:
                    nc.scalar.copy(ob[:], po[:])
                else:
                    nc.vector.tensor_copy(ob[:], po[:])
                nc.sync.dma_start(out[s0:s0 + 128, n * 512:(n + 1) * 512], ob[:])

            def den_chain(h, s_w, ppv, ptsum, at_blk, psc_last):
                # reuse the head's last (already-read) scores tile for the
                # denominator row instead of burning another PSUM slot
                pden = psc_last[0:1, 0:s_w]
                nc.tensor.matmul(pden, lhsT=ones_sb[:],
                                 rhs=ptsum[:, 0:s_w], start=True, stop=True)
                recf = recp.tile([1, 512], F32, tag="recf")
                nc.vector.reciprocal_approx_fast(recf[:, 0:s_w], pden)
                rec = recp.tile([1, 512], BF16, tag="rec")
                nc.vector.tensor_copy(rec[:, 0:s_w], recf[:, 0:s_w])
                rep = recp.tile([128, 512], BF16, tag="rep")
                nc.gpsimd.partition_broadcast(rep[:, 0:s_w], rec[:, 0:s_w])
                nc.vector.tensor_tensor(at_blk[:, h, 0:s_w],
                                        ppv[:, 0:s_w], rep[:, 0:s_w], ALU.mult)

            # (batch, s_lo, s_width); within a batch s ranges must ascend.
            # The final block is split in two halves so half the last
            # out-proj overlaps the second half's attention.
            blocks = [(0, 1536, 512), (0, 1024, 512), (0, 512, 512), (0, 0, 512),
                      (1, 512, 512), (1, 1024, 512), (1, 1536, 512),
                      (1, 0, 256), (1, 256, 256)]
            prev = None
            for bi, (b, s_lo, s_w) in enumerate(blocks):
                # NOTE: keep GpSimd to a single op type (partition_broadcast);
                # mixing in tensor_tensor forces ~5us microcode library swaps.
                acc_eng = nc.vector
                sg = b * SEQ + s_lo
                nt = (s_lo + s_w) // 128             # causal t-tiles
                ntile = HPC * nt
                ngrp_prev = (prev[2] // 128) * 8 if prev is not None else 0
                at_blk = atp.tile([128, HPC, 512], BF16, tag="attnT",
                                  name=f"attnT{sg}")
                emitted = 0
                tidx = 0
                pend = None
                for h in range(HPC):
                    ppv = psA.tile([128, 512], F32, tag="psA", name=f"ppv{sg}{h}")
                    ptsum = accp.tile([128, 512], BF16, tag="ptsum",
                                      name=f"ptsum{sg}{h}")
                    for ti in range(nt):
                        tg = b * SEQ + ti * 128
                        w0 = max(0, 128 * ti - s_lo)
                        diag = 128 * ti >= s_lo
                        psc = psB.tile([128, 512], F32, tag="psB",
                                       name=f"psc{sg}{h}{ti}")
                        nc.tensor.matmul(psc[:, w0:s_w],
                                         lhsT=kT_sb[:, tg:tg + 128],
                                         rhs=qT_sb[:, h, sg + w0:sg + s_w],
                                         start=True, stop=not diag)
                        if diag:                     # additive causal mask via PE
                            nc.tensor.matmul(psc[:, w0:w0 + 128], lhsT=ident[:],
                                             rhs=tri_sb[:],
                                             start=False, stop=True)
                        if ti == 0:
                            pt = ptsum               # exp seeds the running sum
                        else:
                            pt = ptp.tile([128, 512], BF16, tag="pt")
                        nc.scalar.activation(pt[:, w0:s_w], psc[:, w0:s_w], AF.Exp)
                        # out-proj of the previous block goes between scores
                        # and pv so the PE has work during the exp latency.
                        tidx += 1
                        if prev is not None:
                            want = tidx * ngrp_prev // ntile
                            while emitted < want:
                                outproj_group(*prev, emitted)
                                emitted += 1
                        if ti > 0:
                            acc_eng.tensor_tensor(ptsum[:, w0:s_w],
                                                  ptsum[:, w0:s_w],
                                                  pt[:, w0:s_w], ALU.add)
                        nc.tensor.matmul(ppv[:, w0:s_w],
                                         lhsT=v_sb[:, tg // 128, :],
                                         rhs=pt[:, w0:s_w], start=(ti == 0),
                                         stop=(ti == nt - 1))
                    if pend is not None:
                        den_chain(*pend)
                    pend = (h, s_w, ppv, ptsum, at_blk, psc)
                if prev is not None:
                    while emitted < ngrp_prev:
                        outproj_group(*prev, emitted)
                        emitted += 1
                den_chain(*pend)
                if deferred_rope:
                    rope_post(*deferred_rope.pop(0))
                prev = (b, s_lo, s_w, at_blk)
            for g in range((prev[2] // 128) * 8):
                outproj_group(*prev, g)
    nc.finalize()
    return nc


_GRAPH = None


def _get_graph():
    global _GRAPH
    if _GRAPH is None:
        _GRAPH = build_graph()
    return _GRAPH


def prepare_in_maps(x, wq, wk, wv, wo, freqs_cos, freqs_sin, mask, start_pos=0):
    x = np.asarray(x, np.float32)
    wq = np.asarray(wq, np.float32)
    wk = np.asarray(wk, np.float32)
    wv = np.asarray(wv, np.float32)
    wo = np.asarray(wo, np.float32)
    fc = np.asarray(freqs_cos, np.float32)
    fs = np.asarray(freqs_sin, np.float32)

    # evens-first pair permutation (interleaved rope -> rotate-half form)
    perm = np.concatenate([np.arange(0, HD, 2), np.arange(1, HD, 2)])

    def permute_heads(w):
        wr = w.reshape(-1, HD, DIM)[:, perm, :]
        return wr.reshape(-1, DIM)

    wq_p = permute_heads(wq) * (1.0 / math.sqrt(HD))
    wk_p = permute_heads(wk)

    xT = np.ascontiguousarray(x.reshape(BS, DIM).T).astype(NPBF16)
    cosT = np.ascontiguousarray(fc.T).astype(NPBF16)
    sinT = np.ascontiguousarray(fs.T).astype(NPBF16)
    # additive causal triangle for the in-tile diagonal: tri[t, c] = 0 if
    # c >= t else -1e9 (c = column within the 128-wide diagonal strip)
    tt, cc = np.meshgrid(np.arange(128), np.arange(128), indexing="ij")
    tri = np.where(cc >= tt, 0.0, -1e9).astype(NPBF16)

    in_maps = []
    for c in range(NCORES):
        qs = slice(c * HPC * HD, (c + 1) * HPC * HD)
        ks = slice(c * HD, (c + 1) * HD)
        in_maps.append({
            "xT": xT,
            "wqT": np.ascontiguousarray(wq_p[qs, :].T).astype(NPBF16),
            "wkT": np.ascontiguousarray(wk_p[ks, :].T).astype(NPBF16),
            "wvT": np.ascontiguousarray(wv[ks, :].T).astype(NPBF16),
            "woT": np.ascontiguousarray(wo[:, qs].T).astype(NPBF16),
            "cosT": cosT,
            "sinT": sinT,
            "tri": tri,
        })
    return in_maps


def combine_results(results):
    acc = results[0]["out"].astype(np.float64)
    for c in range(1, NCORES):
        acc = acc + results[c]["out"]
    return acc.astype(np.float32).reshape(BSZ, SEQ, DIM)


def run_spmd(in_maps, **kw):
    nc = _get_graph()
    return run_bass_kernel_spmd(nc, in_maps, list(range(NCORES)), **kw)


def kernel(x, wq, wk, wv, wo, freqs_cos, freqs_sin, mask, start_pos=0, **_):
    in_maps = prepare_in_maps(x, wq, wk, wv, wo, freqs_cos, freqs_sin, mask)
    res = run_spmd(in_maps)
    return combine_results(res.results)


# revision 16
# speedup vs baseline: 1.3077x; 1.0538x over previous
"""Tensor-parallel GQA attention prefill (Llama-style) on one TRN2 chip.

Head-sharded across 8 NeuronCores: core c owns q-heads [4c, 4c+4) and
kv-head c.  x is replicated (pre-transposed on host), wq/wk/wv are
column-sharded, wo row-sharded; each core computes a partial output
[B*S, DIM] and the host sums the 8 partials.

Self-contained: shapes hardcoded for
  x[2,2048,4096] wq[4096,4096] wk/wv[1024,4096] wo[4096,4096]
  32 q heads / 8 kv heads / head_dim 128 / causal prefill (start_pos=0).
"""

import math

import numpy as np
import ml_dtypes

import concourse.bass as bass
import concourse.mybir as mybir
from concourse import bacc
from concourse.tile import TileContext
from concourse.bass_utils import run_bass_kernel_spmd
from concourse.masks import make_identity

BSZ, SEQ, DIM = 2, 2048, 4096
NH, NKV, HD = 32, 8, 128
NCORES = 8
HPC = NH // NCORES          # 4 q heads per core
BS = BSZ * SEQ              # 4096 flattened rows
NJ = BS // 512              # 8 s-chunks of 512
KT = DIM // 128             # 32 contraction tiles
SBLK = 4                    # 512-wide s-blocks per batch
BF16 = mybir.dt.bfloat16
F32 = mybir.dt.float32
NPBF16 = ml_dtypes.bfloat16
ALU = mybir.AluOpType
AF = mybir.ActivationFunctionType


def build_graph():
    nc = bacc.Bacc("TRN2", target_bir_lowering=False)
    xT = nc.declare_dram_parameter("xT", [DIM, BS], BF16, isOutput=False)
    wqT = nc.declare_dram_parameter("wqT", [DIM, HPC * HD], BF16, isOutput=False)
    wkT = nc.declare_dram_parameter("wkT", [DIM, HD], BF16, isOutput=False)
    wvT = nc.declare_dram_parameter("wvT", [DIM, HD], BF16, isOutput=False)
    woT = nc.declare_dram_parameter("woT", [HPC * HD, DIM], BF16, isOutput=False)
    cosT = nc.declare_dram_parameter("cosT", [HD // 2, SEQ], BF16, isOutput=False)
    sinT = nc.declare_dram_parameter("sinT", [HD // 2, SEQ], BF16, isOutput=False)
    tri = nc.declare_dram_parameter("tri", [128, 128], BF16, isOutput=False)
    out = nc.declare_dram_parameter("out", [BS, DIM], BF16, isOutput=True)

    with TileContext(nc) as tc:
        with (
            tc.tile_pool(name="const", bufs=1) as const,
            tc.tile_pool(name="xtp", bufs=2) as xtp,
            tc.tile_pool(name="ropep", bufs=2) as ropep,
            tc.tile_pool(name="ptp", bufs=8) as ptp,
            tc.tile_pool(name="atp", bufs=3) as atp,
            tc.tile_pool(name="accp", bufs=2) as accp,
            tc.tile_pool(name="recp", bufs=2) as recp,
            tc.tile_pool(name="osb", bufs=3) as osb,
            tc.tile_pool(name="psA", bufs=4, space="PSUM") as psA,
            tc.tile_pool(name="psB", bufs=4, space="PSUM") as psB,
        ):
            # ---- resident constants / weights -------------------------------
            # wq/wk/wv are DMA'd per k-slice inside the j==0 loop so the
            # first matmuls start as soon as their slice lands.
            wq_sb = const.tile([128, KT, HPC * HD], BF16, tag="wq")
            wk_sb = const.tile([128, KT, HD], BF16, tag="wk")
            wv_sb = const.tile([128, KT, HD], BF16, tag="wv")
            cos_sb = const.tile([64, SEQ], BF16, tag="cos")
            nc.sync.dma_start(cos_sb[:], cosT[:])
            sin_sb = const.tile([64, SEQ], BF16, tag="sin")
            nc.sync.dma_start(sin_sb[:], sinT[:])
            tri_sb = const.tile([128, 128], BF16, tag="tri")
            nc.sync.dma_start(tri_sb[:], tri[:])
            # wo is first needed in the attention phase; loaded there.
            wo_sb = const.tile([128, HPC, DIM], BF16, tag="wo")

            ones_sb = const.tile([128, 1], BF16, tag="ones")
            nc.gpsimd.memset(ones_sb[:], 1.0)
            ident = const.tile([128, 128], BF16, tag="ident")
            make_identity(nc, ident[:])

            # Preload the exp table so the first attention exp doesn't pay
            # the ACT_TABLE_LOAD, and run warm-up matmuls on ident during the
            # initial DMA window so HAM unthrottles before the real work.
            scr = const.tile([128, 1], BF16, tag="scr")
            nc.scalar.activation(scr[:], ones_sb[:], AF.Exp)
            warm = psA.tile([128, 512], F32, tag="psA", name="warm")
            for _ in range(130):
                nc.tensor.matmul(warm[:, 0:128], lhsT=ident[:], rhs=ident[:],
                                 start=True, stop=True)

            # ---- resident activations ---------------------------------------
            qT_sb = const.tile([128, HPC, BS], BF16, tag="qT")    # per-head Q^T
            kT_sb = const.tile([128, BS], BF16, tag="kT")         # K^T (d, t)
            v_sb = const.tile([128, BS // 128, HD], BF16, tag="v")  # V (t, d) tiles
            # attention output lives per-block in the atp pool (2 blocks live)

            def rope_pre(psum):
                """Copy psum halves to SBUF (releases the PSUM slot)."""
                te = ropep.tile([64, 512], BF16, tag="ropetmpe", bufs=5)
                to = ropep.tile([64, 512], BF16, tag="ropetmpo", bufs=5)
                nc.scalar.copy(te[:], psum[0:64])
                nc.vector.tensor_copy(to[:], psum[64:128])
                return te, to

            def rope_post(te, to, dst, soff):
                cs = cos_sb[:, soff:soff + 512]
                sn = sin_sb[:, soff:soff + 512]
                t1 = ropep.tile([64, 512], BF16, tag="t1")
                t2 = ropep.tile([64, 512], BF16, tag="t2")
                nc.vector.tensor_tensor(t1[:], te[:], cs, ALU.mult)
                nc.vector.tensor_tensor(t2[:], to[:], sn, ALU.mult)
                nc.vector.tensor_tensor(dst[0:64], t1[:], t2[:], ALU.subtract)
                t3 = ropep.tile([64, 512], BF16, tag="t1")
                t4 = ropep.tile([64, 512], BF16, tag="t2")
                nc.vector.tensor_tensor(t3[:], te[:], sn, ALU.mult)
                nc.vector.tensor_tensor(t4[:], to[:], cs, ALU.mult)
                nc.vector.tensor_tensor(dst[64:128], t3[:], t4[:], ALU.add)

            def rope_copy(psum, dst, soff):
                """psum [128,512] (evens-first layout) -> rotated bf16 dst."""
                te, to = rope_pre(psum)
                rope_post(te, to, dst, soff)

            deferred_rope = []

            # ================= Phase 1: QKV projection =======================
            # single pass over xT per s-chunk: 4 Q accumulators in psA,
            # K/V accumulators in psB.  The Q chains trail K/V by QLAG
            # k-steps so at chunk boundaries the K/V chains of chunk j+1
            # start while chunk j's Q rope copies drain the psA banks.
            for j in range(NJ):
                soff = (j % SBLK) * 512      # within-batch s offset
                js = slice(j * 512, (j + 1) * 512)
                qps = [psA.tile([128, 512], F32, tag="psA", name=f"qps{j}_{c}") for c in range(HPC)]
                kp = psB.tile([128, 512], F32, tag="psB", name=f"kp{j}")
                vp = psB.tile([128, 512], F32, tag="psB", name=f"vp{j}")
                xts = {}

                for kc in range(KT // 8):
                    # 1MB x chunks + 8-k-tile weight chunks: DIRECT2D issue
                    # on the Sync sequencer costs ~600ns per dma_start, so
                    # descriptor COUNT (not HBM bandwidth) throttles phase-1
                    # startup if the transfers are small.
                    xt = xtp.tile([128, 8, 512], BF16, tag="xt")
                    if j == 0:
                        ks = slice(kc * 1024, (kc + 1) * 1024)
                        kt8 = slice(kc * 8, (kc + 1) * 8)
                        nc.sync.dma_start(
                            wq_sb[:, kt8, :],
                            wqT[ks, :].rearrange("(a p) m -> p a m", p=128))
                        nc.sync.dma_start(
                            wk_sb[:, kt8, :],
                            wkT[ks, :].rearrange("(a p) m -> p a m", p=128))
                        nc.sync.dma_start(
                            wv_sb[:, kt8, :],
                            wvT[ks, :].rearrange("(a p) m -> p a m", p=128))
                    nc.sync.dma_start(
                        xt[:],
                        xT[kc * 1024:(kc + 1) * 1024, js].rearrange("(a p) m -> p a m", p=128))
                    for k8 in range(8):
                        k = kc * 8 + k8
                        nc.tensor.matmul(kp[:], lhsT=wk_sb[:, k, :], rhs=xt[:, k8, :],
                                         start=(k == 0), stop=(k == KT - 1))
                        nc.tensor.matmul(vp[:], lhsT=wv_sb[:, k, :], rhs=xt[:, k8, :],
                                         start=(k == 0), stop=(k == KT - 1))
                        for c in range(HPC):
                            nc.tensor.matmul(
                                qps[c][:], lhsT=wq_sb[:, k, c * 128:(c + 1) * 128],
                                rhs=xt[:, k8, :], start=(k == 0), stop=(k == KT - 1))
                # K/V first: attention needs them (and their PSUM slots) at the
                # phase boundary before any Q-rope results.
                rope_copy(kp, kT_sb[:, js], soff)
                # V^T chunk -> natural-layout V tiles via DMA XBAR transpose.
                # Last chunk's PSUM copy goes on DVE so the ScalarE queue is
                # clear for the first attention exp at the phase boundary.
                last = j == NJ - 1
                vtmp = ropep.tile([128, 512], BF16, tag="vtmp")
                if last:
                    nc.vector.tensor_copy(vtmp[:], vp[:])
                else:
                    nc.scalar.copy(vtmp[:], vp[:])
                nc.sync.dma_start_transpose(v_sb[:, j * 4:(j + 1) * 4, :], vtmp[:])
                for c in range(HPC):
                    if last:
                        # defer the DVE rope chains past the phase boundary;
                        # qT chunk 7 isn't read until the 7th attention block.
                        te, to = rope_pre(qps[c])
                        deferred_rope.append((te, to, qT_sb[:, c, js], soff))
                    else:
                        rope_copy(qps[c], qT_sb[:, c, js], soff)
                if j == 4:
                    # mid-phase: DMA bandwidth has headroom here and wo is
                    # needed right after the phase boundary.
                    nc.sync.dma_start(wo_sb[:], woT.rearrange("(a p) m -> p a m", p=128))

            # ================= Phase 2+3: attention + out-proj ===============
            # Out-proj of the previous block is interleaved at attention-tile
            # granularity so the PE never starves while ScalarE runs exp; the
            # den/recip chain of each head is deferred by one head so its
            # den-matmul never blocks the PE FIFO on the DVE accumulation.
            def outproj_group(b, s_lo, s_w, at_blk, g):
                st, n = g // 8, g % 8
                s0 = b * SEQ + s_lo + st * 128
                po = psA.tile([128, 512], F32, tag="psA", name=f"po{b}_{s_lo}_{g}")
                for dt in range(HPC):
                    nc.tensor.matmul(
                        po[:], lhsT=at_blk[:, dt, st * 128:(st + 1) * 128],
                        rhs=wo_sb[:, dt, n * 512:(n + 1) * 512],
                        start=(dt == 0), stop=(dt == HPC - 1))
                ob = osb.tile([128, 512], BF16, tag="ob")
                if n % 2 == 0:
                    nc.scalar.copy(ob[:], po[:])
                else:
                    nc.vector.tensor_copy(ob[:], po[:])
                nc.sync.dma_start(out[s0:s0 + 128, n * 512:(n + 1) * 512], ob[:])

            def den_chain(h, s_w, ppv, ptsum, at_blk, psc_last):
                # reuse the head's last (already-read) scores tile for the
                # denominator row instead of burning another PSUM slot
                pden = psc_last[0:1, 0:s_w]
                nc.tensor.matmul(pden, lhsT=ones_sb[:],
                                 rhs=ptsum[:, 0:s_w], start=True, stop=True)
                recf = recp.tile([1, 512], F32, tag="recf")
                nc.vector.reciprocal_approx_fast(recf[:, 0:s_w], pden)
                rec = recp.tile([1, 512], BF16, tag="rec")
                nc.vector.tensor_copy(rec[:, 0:s_w], recf[:, 0:s_w])
                rep = recp.tile([128, 512], BF16, tag="rep")
                nc.gpsimd.partition_broadcast(rep[:, 0:s_w], rec[:, 0:s_w])
                nc.vector.tensor_tensor(at_blk[:, h, 0:s_w],
                                        ppv[:, 0:s_w], rep[:, 0:s_w], ALU.mult)

            # (batch, s_lo, s_width); within a batch s ranges must ascend.
            # The final block is split in two halves so half the last
            # out-proj overlaps the second half's attention.
            blocks = [(0, 0, 512), (0, 512, 512), (0, 1024, 512), (0, 1536, 512),
                      (1, 512, 512), (1, 1024, 512), (1, 1536, 512),
                      (1, 0, 256), (1, 256, 256)]
            prev = None
            for bi, (b, s_lo, s_w) in enumerate(blocks):
                # NOTE: keep GpSimd to a single op type (partition_broadcast);
                # mixing in tensor_tensor forces ~5us microcode library swaps.
                acc_eng = nc.vector
                sg = b * SEQ + s_lo
                nt = (s_lo + s_w) // 128             # causal t-tiles
                ntile = HPC * nt
                ngrp_prev = (prev[2] // 128) * 8 if prev is not None else 0
                at_blk = atp.tile([128, HPC, 512], BF16, tag="attnT",
                                  name=f"attnT{sg}")
                emitted = 0
                tidx = 0
                pend = None
                for h in range(HPC):
                    ppv = psA.tile([128, 512], F32, tag="psA", name=f"ppv{sg}{h}")
                    ptsum = accp.tile([128, 512], BF16, tag="ptsum",
                                      name=f"ptsum{sg}{h}")
                    for ti in range(nt):
                        tg = b * SEQ + ti * 128
                        w0 = max(0, 128 * ti - s_lo)
                        diag = 128 * ti >= s_lo
                        psc = psB.tile([128, 512], F32, tag="psB",
                                       name=f"psc{sg}{h}{ti}")
                        nc.tensor.matmul(psc[:, w0:s_w],
                                         lhsT=kT_sb[:, tg:tg + 128],
                                         rhs=qT_sb[:, h, sg + w0:sg + s_w],
                                         start=True, stop=not diag)
                        if diag:                     # additive causal mask via PE
                            nc.tensor.matmul(psc[:, w0:w0 + 128], lhsT=ident[:],
                                             rhs=tri_sb[:],
                                             start=False, stop=True)
                        if ti == 0:
                            pt = ptsum               # exp seeds the running sum
                        else:
                            pt = ptp.tile([128, 512], BF16, tag="pt")
                        nc.scalar.activation(pt[:, w0:s_w], psc[:, w0:s_w], AF.Exp)
                        # out-proj of the previous block goes between scores
                        # and pv so the PE has work during the exp latency.
                        tidx += 1
                        if prev is not None:
                            want = tidx * ngrp_prev // ntile
                            while emitted < want:
                                outproj_group(*prev, emitted)
                                emitted += 1
                        if ti > 0:
                            acc_eng.tensor_tensor(ptsum[:, w0:s_w],
                                                  ptsum[:, w0:s_w],
                                                  pt[:, w0:s_w], ALU.add)
                        nc.tensor.matmul(ppv[:, w0:s_w],
                                         lhsT=v_sb[:, tg // 128, :],
                                         rhs=pt[:, w0:s_w], start=(ti == 0),
                                         stop=(ti == nt - 1))
                    if pend is not None:
                        den_chain(*pend)
                    pend = (h, s_w, ppv, ptsum, at_blk, psc)
                if prev is not None:
                    while emitted < ngrp_prev:
                        outproj_group(*prev, emitted)
                        emitted += 1
                den_chain(*pend)
                if deferred_rope:
                    rope_post(*deferred_rope.pop(0))
                prev = (b, s_lo, s_w, at_blk)
            for g in range((prev[2] // 128) * 8):
                outproj_group(*prev, g)
    nc.finalize()
    return nc


_GRAPH = None


def _get_graph():
    global _GRAPH
    if _GRAPH is None:
        _GRAPH = build_graph()
    return _GRAPH


def prepare_in_maps(x, wq, wk, wv, wo, freqs_cos, freqs_sin, mask, start_pos=0):
    x = np.asarray(x, np.float32)
    wq = np.asarray(wq, np.float32)
    wk = np.asarray(wk, np.float32)
    wv = np.asarray(wv, np.float32)
    wo = np.asarray(wo, np.float32)
    fc = np.asarray(freqs_cos, np.float32)
    fs = np.asarray(freqs_sin, np.float32)

    # evens-first pair permutation (interleaved rope -> rotate-half form)
    perm = np.concatenate([np.arange(0, HD, 2), np.arange(1, HD, 2)])

    def permute_heads(w):
        wr = w.reshape(-1, HD, DIM)[:, perm, :]
        return wr.reshape(-1, DIM)

    wq_p = permute_heads(wq) * (1.0 / math.sqrt(HD))
    wk_p = permute_heads(wk)

    xT = np.ascontiguousarray(x.reshape(BS, DIM).T).astype(NPBF16)
    cosT = np.ascontiguousarray(fc.T).astype(NPBF16)
    sinT = np.ascontiguousarray(fs.T).astype(NPBF16)
    # additive causal triangle for the in-tile diagonal: tri[t, c] = 0 if
    # c >= t else -1e9 (c = column within the 128-wide diagonal strip)
    tt, cc = np.meshgrid(np.arange(128), np.arange(128), indexing="ij")
    tri = np.where(cc >= tt, 0.0, -1e9).astype(NPBF16)

    in_maps = []
    for c in range(NCORES):
        qs = slice(c * HPC * HD, (c + 1) * HPC * HD)
        ks = slice(c * HD, (c + 1) * HD)
        in_maps.append({
            "xT": xT,
            "wqT": np.ascontiguousarray(wq_p[qs, :].T).astype(NPBF16),
            "wkT": np.ascontiguousarray(wk_p[ks, :].T).astype(NPBF16),
            "wvT": np.ascontiguousarray(wv[ks, :].T).astype(NPBF16),
            "woT": np.ascontiguousarray(wo[:, qs].T).astype(NPBF16),
            "cosT": cosT,
            "sinT": sinT,
            "tri": tri,
        })
    return in_maps


def combine_results(results):
    acc = results[0]["out"].astype(np.float64)
    for c in range(1, NCORES):
        acc = acc + results[c]["out"]
    return acc.astype(np.float32).reshape(BSZ, SEQ, DIM)


def run_spmd(in_maps, **kw):
    nc = _get_graph()
    return run_bass_kernel_spmd(nc, in_maps, list(range(NCORES)), **kw)


def kernel(x, wq, wk, wv, wo, freqs_cos, freqs_sin, mask, start_pos=0, **_):
    in_maps = prepare_in_maps(x, wq, wk, wv, wo, freqs_cos, freqs_sin, mask)
    res = run_spmd(in_maps)
    return combine_results(res.results)


# revision 17
# speedup vs baseline: 1.3123x; 1.0035x over previous
"""Tensor-parallel GQA attention prefill (Llama-style) on one TRN2 chip.

Head-sharded across 8 NeuronCores: core c owns q-heads [4c, 4c+4) and
kv-head c.  x is replicated (pre-transposed on host), wq/wk/wv are
column-sharded, wo row-sharded; each core computes a partial output
[B*S, DIM] and the host sums the 8 partials.

Self-contained: shapes hardcoded for
  x[2,2048,4096] wq[4096,4096] wk/wv[1024,4096] wo[4096,4096]
  32 q heads / 8 kv heads / head_dim 128 / causal prefill (start_pos=0).
"""

import math

import numpy as np
import ml_dtypes

import concourse.bass as bass
import concourse.mybir as mybir
from concourse import bacc
from concourse.tile import TileContext
from concourse.bass_utils import run_bass_kernel_spmd
from concourse.masks import make_identity

BSZ, SEQ, DIM = 2, 2048, 4096
NH, NKV, HD = 32, 8, 128
NCORES = 8
HPC = NH // NCORES          # 4 q heads per core
BS = BSZ * SEQ              # 4096 flattened rows
NJ = BS // 512              # 8 s-chunks of 512
KT = DIM // 128             # 32 contraction tiles
SBLK = 4                    # 512-wide s-blocks per batch
BF16 = mybir.dt.bfloat16
F32 = mybir.dt.float32
NPBF16 = ml_dtypes.bfloat16
ALU = mybir.AluOpType
AF = mybir.ActivationFunctionType


def build_graph():
    nc = bacc.Bacc("TRN2", target_bir_lowering=False)
    xT = nc.declare_dram_parameter("xT", [DIM, BS], BF16, isOutput=False)
    wqT = nc.declare_dram_parameter("wqT", [DIM, HPC * HD], BF16, isOutput=False)
    wkT = nc.declare_dram_parameter("wkT", [DIM, HD], BF16, isOutput=False)
    wvT = nc.declare_dram_parameter("wvT", [DIM, HD], BF16, isOutput=False)
    woT = nc.declare_dram_parameter("woT", [HPC * HD, DIM], BF16, isOutput=False)
    cosT = nc.declare_dram_parameter("cosT", [HD // 2, SEQ], BF16, isOutput=False)
    sinT = nc.declare_dram_parameter("sinT", [HD // 2, SEQ], BF16, isOutput=False)
    tri = nc.declare_dram_parameter("tri", [128, 128], BF16, isOutput=False)
    out = nc.declare_dram_parameter("out", [BS, DIM], BF16, isOutput=True)

    with TileContext(nc) as tc:
        with (
            tc.tile_pool(name="const", bufs=1) as const,
            tc.tile_pool(name="xtp", bufs=2) as xtp,
            tc.tile_pool(name="ropep", bufs=2) as ropep,
            tc.tile_pool(name="ptp", bufs=8) as ptp,
            tc.tile_pool(name="atp", bufs=3) as atp,
            tc.tile_pool(name="accp", bufs=2) as accp,
            tc.tile_pool(name="recp", bufs=2) as recp,
            tc.tile_pool(name="osb", bufs=3) as osb,
            tc.tile_pool(name="psA", bufs=5, space="PSUM") as psA,
            tc.tile_pool(name="psB", bufs=3, space="PSUM") as psB,
        ):
            # ---- resident constants / weights -------------------------------
            # wq/wk/wv are DMA'd per k-slice inside the j==0 loop so the
            # first matmuls start as soon as their slice lands.
            wq_sb = const.tile([128, KT, HPC * HD], BF16, tag="wq")
            wk_sb = const.tile([128, KT, HD], BF16, tag="wk")
            wv_sb = const.tile([128, KT, HD], BF16, tag="wv")
            cos_sb = const.tile([64, SEQ], BF16, tag="cos")
            nc.sync.dma_start(cos_sb[:], cosT[:])
            sin_sb = const.tile([64, SEQ], BF16, tag="sin")
            nc.sync.dma_start(sin_sb[:], sinT[:])
            tri_sb = const.tile([128, 128], BF16, tag="tri")
            nc.sync.dma_start(tri_sb[:], tri[:])
            # wo is first needed in the attention phase; loaded there.
            wo_sb = const.tile([128, HPC, DIM], BF16, tag="wo")

            ones_sb = const.tile([128, 1], BF16, tag="ones")
            nc.gpsimd.memset(ones_sb[:], 1.0)
            ident = const.tile([128, 128], BF16, tag="ident")
            make_identity(nc, ident[:])

            # Preload the exp table so the first attention exp doesn't pay
            # the ACT_TABLE_LOAD, and run warm-up matmuls on ident during the
            # initial DMA window so HAM unthrottles before the real work.
            scr = const.tile([128, 1], BF16, tag="scr")
            nc.scalar.activation(scr[:], ones_sb[:], AF.Exp)
            warm = psA.tile([128, 512], F32, tag="psA", name="warm")
            for _ in range(130):
                nc.tensor.matmul(warm[:, 0:128], lhsT=ident[:], rhs=ident[:],
                                 start=True, stop=True)

            # ---- resident activations ---------------------------------------
            qT_sb = const.tile([128, HPC, BS], BF16, tag="qT")    # per-head Q^T
            kT_sb = const.tile([128, BS], BF16, tag="kT")         # K^T (d, t)
            v_sb = const.tile([128, BS // 128, HD], BF16, tag="v")  # V (t, d) tiles
            # attention output lives per-block in the atp pool (2 blocks live)

            def rope_pre(psum):
                """Copy psum halves to SBUF (releases the PSUM slot)."""
                te = ropep.tile([64, 512], BF16, tag="ropetmpe", bufs=5)
                to = ropep.tile([64, 512], BF16, tag="ropetmpo", bufs=5)
                nc.scalar.copy(te[:], psum[0:64])
                nc.vector.tensor_copy(to[:], psum[64:128])
                return te, to

            def rope_post(te, to, dst, soff):
                cs = cos_sb[:, soff:soff + 512]
                sn = sin_sb[:, soff:soff + 512]
                t1 = ropep.tile([64, 512], BF16, tag="t1")
                t2 = ropep.tile([64, 512], BF16, tag="t2")
                nc.vector.tensor_tensor(t1[:], te[:], cs, ALU.mult)
                nc.vector.tensor_tensor(t2[:], to[:], sn, ALU.mult)
                nc.vector.tensor_tensor(dst[0:64], t1[:], t2[:], ALU.subtract)
                t3 = ropep.tile([64, 512], BF16, tag="t1")
                t4 = ropep.tile([64, 512], BF16, tag="t2")
                nc.vector.tensor_tensor(t3[:], te[:], sn, ALU.mult)
                nc.vector.tensor_tensor(t4[:], to[:], cs, ALU.mult)
                nc.vector.tensor_tensor(dst[64:128], t3[:], t4[:], ALU.add)

            def rope_copy(psum, dst, soff):
                """psum [128,512] (evens-first layout) -> rotated bf16 dst."""
                te, to = rope_pre(psum)
                rope_post(te, to, dst, soff)

            deferred_rope = []

            # ================= Phase 1: QKV projection =======================
            # single pass over xT per s-chunk: 4 Q accumulators in psA,
            # K/V accumulators in psB.  The Q chains trail K/V by QLAG
            # k-steps so at chunk boundaries the K/V chains of chunk j+1
            # start while chunk j's Q rope copies drain the psA banks.
            for j in range(NJ):
                soff = (j % SBLK) * 512      # within-batch s offset
                js = slice(j * 512, (j + 1) * 512)
                qps = [psA.tile([128, 512], F32, tag="psA", name=f"qps{j}_{c}") for c in range(HPC)]
                kp = psB.tile([128, 512], F32, tag="psB", name=f"kp{j}")
                vp = psB.tile([128, 512], F32, tag="psB", name=f"vp{j}")
                xts = {}

                for kc in range(KT // 8):
                    # 1MB x chunks + 8-k-tile weight chunks: DIRECT2D issue
                    # on the Sync sequencer costs ~600ns per dma_start, so
                    # descriptor COUNT (not HBM bandwidth) throttles phase-1
                    # startup if the transfers are small.
                    xt = xtp.tile([128, 8, 512], BF16, tag="xt")
                    if j == 0:
                        ks = slice(kc * 1024, (kc + 1) * 1024)
                        kt8 = slice(kc * 8, (kc + 1) * 8)
                        nc.sync.dma_start(
                            wq_sb[:, kt8, :],
                            wqT[ks, :].rearrange("(a p) m -> p a m", p=128))
                        nc.sync.dma_start(
                            wk_sb[:, kt8, :],
                            wkT[ks, :].rearrange("(a p) m -> p a m", p=128))
                        nc.sync.dma_start(
                            wv_sb[:, kt8, :],
                            wvT[ks, :].rearrange("(a p) m -> p a m", p=128))
                    nc.sync.dma_start(
                        xt[:],
                        xT[kc * 1024:(kc + 1) * 1024, js].rearrange("(a p) m -> p a m", p=128))
                    for k8 in range(8):
                        k = kc * 8 + k8
                        nc.tensor.matmul(kp[:], lhsT=wk_sb[:, k, :], rhs=xt[:, k8, :],
                                         start=(k == 0), stop=(k == KT - 1))
                        nc.tensor.matmul(vp[:], lhsT=wv_sb[:, k, :], rhs=xt[:, k8, :],
                                         start=(k == 0), stop=(k == KT - 1))
                        for c in range(HPC):
                            nc.tensor.matmul(
                                qps[c][:], lhsT=wq_sb[:, k, c * 128:(c + 1) * 128],
                                rhs=xt[:, k8, :], start=(k == 0), stop=(k == KT - 1))
                # K/V first: attention needs them (and their PSUM slots) at the
                # phase boundary before any Q-rope results.
                rope_copy(kp, kT_sb[:, js], soff)
                # V^T chunk -> natural-layout V tiles via DMA XBAR transpose.
                # Last chunk's PSUM copy goes on DVE so the ScalarE queue is
                # clear for the first attention exp at the phase boundary.
                last = j == NJ - 1
                vtmp = ropep.tile([128, 512], BF16, tag="vtmp")
                if last:
                    nc.vector.tensor_copy(vtmp[:], vp[:])
                else:
                    nc.scalar.copy(vtmp[:], vp[:])
                nc.sync.dma_start_transpose(v_sb[:, j * 4:(j + 1) * 4, :], vtmp[:])
                for c in range(HPC):
                    if last:
                        # defer the DVE rope chains past the phase boundary;
                        # qT chunk 7 isn't read until the 7th attention block.
                        te, to = rope_pre(qps[c])
                        deferred_rope.append((te, to, qT_sb[:, c, js], soff))
                    else:
                        rope_copy(qps[c], qT_sb[:, c, js], soff)
                if j == 4:
                    # mid-phase: DMA bandwidth has headroom here and wo is
                    # needed right after the phase boundary.
                    nc.sync.dma_start(wo_sb[:], woT.rearrange("(a p) m -> p a m", p=128))

            # ================= Phase 2+3: attention + out-proj ===============
            # Out-proj of the previous block is interleaved at attention-tile
            # granularity so the PE never starves while ScalarE runs exp; the
            # den/recip chain of each head is deferred by one head so its
            # den-matmul never blocks the PE FIFO on the DVE accumulation.
            def outproj_group(b, s_lo, s_w, at_blk, g):
                st, n = g // 8, g % 8
                s0 = b * SEQ + s_lo + st * 128
                po = psA.tile([128, 512], F32, tag="psA", name=f"po{b}_{s_lo}_{g}")
                for dt in range(HPC):
                    nc.tensor.matmul(
                        po[:], lhsT=at_blk[:, dt, st * 128:(st + 1) * 128],
                        rhs=wo_sb[:, dt, n * 512:(n + 1) * 512],
                        start=(dt == 0), stop=(dt == HPC - 1))
                ob = osb.tile([128, 512], BF16, tag="ob")
                if n % 2 == 0:
                    nc.scalar.copy(ob[:], po[:])
                else:
                    nc.vector.tensor_copy(ob[:], po[:])
                nc.sync.dma_start(out[s0:s0 + 128, n * 512:(n + 1) * 512], ob[:])

            def den_chain(h, s_w, ppv, ptsum, at_blk, psc_last):
                # reuse the head's last (already-read) scores tile for the
                # denominator row instead of burning another PSUM slot
                pden = psc_last[0:1, 0:s_w]
                nc.tensor.matmul(pden, lhsT=ones_sb[:],
                                 rhs=ptsum[:, 0:s_w], start=True, stop=True)
                recf = recp.tile([1, 512], F32, tag="recf")
                nc.vector.reciprocal_approx_fast(recf[:, 0:s_w], pden)
                rec = recp.tile([1, 512], BF16, tag="rec")
                nc.vector.tensor_copy(rec[:, 0:s_w], recf[:, 0:s_w])
                rep = recp.tile([128, 512], BF16, tag="rep")
                nc.gpsimd.partition_broadcast(rep[:, 0:s_w], rec[:, 0:s_w])
                nc.vector.tensor_tensor(at_blk[:, h, 0:s_w],
                                        ppv[:, 0:s_w], rep[:, 0:s_w], ALU.mult)

            # (batch, s_lo, s_width); within a batch s ranges must ascend.
            # The final block is split in two halves so half the last
            # out-proj overlaps the second half's attention.
            blocks = [(0, 0, 512), (0, 512, 512), (0, 1024, 512), (0, 1536, 512),
                      (1, 512, 512), (1, 1024, 512), (1, 1536, 512),
                      (1, 0, 256), (1, 256, 256)]
            prev = None
            for bi, (b, s_lo, s_w) in enumerate(blocks):
                # NOTE: keep GpSimd to a single op type (partition_broadcast);
                # mixing in tensor_tensor forces ~5us microcode library swaps.
                acc_eng = nc.vector
                sg = b * SEQ + s_lo
                nt = (s_lo + s_w) // 128             # causal t-tiles
                ntile = HPC * nt
                ngrp_prev = (prev[2] // 128) * 8 if prev is not None else 0
                at_blk = atp.tile([128, HPC, 512], BF16, tag="attnT",
                                  name=f"attnT{sg}")
                emitted = 0
                tidx = 0
                pend = None
                for h in range(HPC):
                    ppv = psA.tile([128, 512], F32, tag="psA", name=f"ppv{sg}{h}")
                    ptsum = accp.tile([128, 512], BF16, tag="ptsum",
                                      name=f"ptsum{sg}{h}")
                    for ti in range(nt):
                        tg = b * SEQ + ti * 128
                        w0 = max(0, 128 * ti - s_lo)
                        diag = 128 * ti >= s_lo
                        psc = psB.tile([128, 512], F32, tag="psB",
                                       name=f"psc{sg}{h}{ti}")
                        nc.tensor.matmul(psc[:, w0:s_w],
                                         lhsT=kT_sb[:, tg:tg + 128],
                                         rhs=qT_sb[:, h, sg + w0:sg + s_w],
                                         start=True, stop=not diag)
                        if diag:                     # additive causal mask via PE
                            nc.tensor.matmul(psc[:, w0:w0 + 128], lhsT=ident[:],
                                             rhs=tri_sb[:],
                                             start=False, stop=True)
                        if ti == 0:
                            pt = ptsum               # exp seeds the running sum
                        else:
                            pt = ptp.tile([128, 512], BF16, tag="pt")
                        nc.scalar.activation(pt[:, w0:s_w], psc[:, w0:s_w], AF.Exp)
                        # out-proj of the previous block goes between scores
                        # and pv so the PE has work during the exp latency.
                        tidx += 1
                        if prev is not None:
                            want = tidx * ngrp_prev // ntile
                            while emitted < want:
                                outproj_group(*prev, emitted)
                                emitted += 1
                        if ti > 0:
                            acc_eng.tensor_tensor(ptsum[:, w0:s_w],
                                                  ptsum[:, w0:s_w],
                                                  pt[:, w0:s_w], ALU.add)
                        nc.tensor.matmul(ppv[:, w0:s_w],
                                         lhsT=v_sb[:, tg // 128, :],
                                         rhs=pt[:, w0:s_w], start=(ti == 0),
                                         stop=(ti == nt - 1))
                    if pend is not None:
                        den_chain(*pend)
                    pend = (h, s_w, ppv, ptsum, at_blk, psc)
                if prev is not None:
                    while emitted < ngrp_prev:
                        outproj_group(*prev, emitted)
                        emitted += 1
                den_chain(*pend)
                if deferred_rope:
                    rope_post(*deferred_rope.pop(0))
                prev = (b, s_lo, s_w, at_blk)
            for g in range((prev[2] // 128) * 8):
                outproj_group(*prev, g)
    nc.finalize()
    return nc


_GRAPH = None


def _get_graph():
    global _GRAPH
    if _GRAPH is None:
        _GRAPH = build_graph()
    return _GRAPH


def prepare_in_maps(x, wq, wk, wv, wo, freqs_cos, freqs_sin, mask, start_pos=0):
    x = np.asarray(x, np.float32)
    wq = np.asarray(wq, np.float32)
    wk = np.asarray(wk, np.float32)
    wv = np.asarray(wv, np.float32)
    wo = np.asarray(wo, np.float32)
    fc = np.asarray(freqs_cos, np.float32)
    fs = np.asarray(freqs_sin, np.float32)

    # evens-first pair permutation (interleaved rope -> rotate-half form)
    perm = np.concatenate([np.arange(0, HD, 2), np.arange(1, HD, 2)])

    def permute_heads(w):
        wr = w.reshape(-1, HD, DIM)[:, perm, :]
        return wr.reshape(-1, DIM)

    wq_p = permute_heads(wq) * (1.0 / math.sqrt(HD))
    wk_p = permute_heads(wk)

    xT = np.ascontiguousarray(x.reshape(BS, DIM).T).astype(NPBF16)
    cosT = np.ascontiguousarray(fc.T).astype(NPBF16)
    sinT = np.ascontiguousarray(fs.T).astype(NPBF16)
    # additive causal triangle for the in-tile diagonal: tri[t, c] = 0 if
    # c >= t else -1e9 (c = column within the 128-wide diagonal strip)
    tt, cc = np.meshgrid(np.arange(128), np.arange(128), indexing="ij")
    tri = np.where(cc >= tt, 0.0, -1e9).astype(NPBF16)

    in_maps = []
    for c in range(NCORES):
        qs = slice(c * HPC * HD, (c + 1) * HPC * HD)
        ks = slice(c * HD, (c + 1) * HD)
        in_maps.append({
            "xT": xT,
            "wqT": np.ascontiguousarray(wq_p[qs, :].T).astype(NPBF16),
            "wkT": np.ascontiguousarray(wk_p[ks, :].T).astype(NPBF16),
            "wvT": np.ascontiguousarray(wv[ks, :].T).astype(NPBF16),
            "woT": np.ascontiguousarray(wo[:, qs].T).astype(NPBF16),
            "cosT": cosT,
            "sinT": sinT,
            "tri": tri,
        })
    return in_maps


def combine_results(results):
    acc = results[0]["out"].astype(np.float64)
    for c in range(1, NCORES):
        acc = acc + results[c]["out"]
    return acc.astype(np.float32).reshape(BSZ, SEQ, DIM)


def run_spmd(in_maps, **kw):
    nc = _get_graph()
    return run_bass_kernel_spmd(nc, in_maps, list(range(NCORES)), **kw)


def kernel(x, wq, wk, wv, wo, freqs_cos, freqs_sin, mask, start_pos=0, **_):
    in_maps = prepare_in_maps(x, wq, wk, wv, wo, freqs_cos, freqs_sin, mask)
    res = run_spmd(in_maps)
    return combine_results(res.results)
